# revision 37
# baseline (speedup 1.0000x reference)
"""CrossAttention2D Trainium2 Bass kernel.

Problem (per batch item b, C=128, HW=64*64=4096):
    q = Wq @ xq + bq            # [C, HW]   (1x1 conv == GEMM)
    k = Wk @ xk + bk            # [C, HW]
    S = (q^T k) / sqrt(HW)      # [HW, HW]
    A = softmax(S, axis=-1)
    out = (A @ v^T)^T + q       # [C, HW],  v = xv
Sharding: data-parallel over batch B=8 -> one batch item per NeuronCore.

Per-core schedule (ACT exp stream is the bottleneck: 128 ops x ~1.06us):
  - Inputs are pre-cast to bf16 on the HOST (halves DMA bytes, kills all
    on-device input casts; projections were bf16 anyway).
  - Scores computed TRANSPOSED: S^T tiles [tk=128, tq=1024], exp on
    ScalarE with FD=1024 (2 PSUM banks, ring of 2).
  - Software pipeline per iteration g: emit scores/exp for block g FIRST,
    then trickled work (projections, finalize), then PV batches that lag
    LAG blocks behind - so ScalarE never waits.
  - PV: out_ext[tq,129] += expS^T_slice^T @ vT_ext (ones column gives the
    softmax denominator for free), accumulated over 32 key blocks in 3
    PSUM banks (3 accumulators packed per bank).
  - Last SCH columns of each exp tile for chunks 1-3 computed on the DVE
    via the Schraudolph bit-trick (one fused mul-add-convert to int16 ==
    the bf16 pattern of exp) - offloads ~12% of the exp stream from the
    bottleneck ScalarE; softmax cancels the approximation bias.
  - vT: slab 0 by PE transposes (PE is idle pre-PV), slabs 1-3 by 16-bit
    DMA-transpose into a dense buffer, DVE copies bridge to the
    ones-column layout. Chunk 0's PV lag starts at 9 (slabs land late).
  - Chunk boundaries: V emission pauses 3 iterations so pass1 (recip +
    normalize on DVE) can drain the o-banks before the next chunk's
    first PV (start=True on the first write per bank replaces zero-init
    matmuls); V catches back up with double-emission iterations.
  - Bulk input DMAs are held back behind the head-critical transfers via
    gpsimd WAR-reads of their destination regions (a plain emission-order
    gate gets reordered by the tile scheduler).
  - Tail: finalize of the last chunk splits normalize between the (idle)
    ScalarE and DVE; chunked output stores.
"""

import os
import numpy as np

B, C, H, W = 8, 128, 64, 64
HW = H * W            # 4096
P = 128
TQ = 512              # one score matmul / one PSUM bank
TQC = 1024            # query-token chunk (2 banks -> one FD=1024 exp)
NCHUNK = HW // TQC    # 4
NTK = HW // P         # 32 key blocks
NBLK = NCHUNK * NTK   # 128 score/exp groups
VT_STRIDE = 130       # 129 used + 1 pad (4B alignment per block)
LAG = 5               # PV lags scores/exp by this many blocks

_CACHE: dict = {}
LAST_RESULTS = None   # BassKernelResults of the most recent run (for test.py)


def _build_kernel():
    import concourse.tile as tile
    from concourse import bacc, mybir
    from concourse.masks import make_identity

    f32 = mybir.dt.float32
    bf16 = mybir.dt.bfloat16
    AF = mybir.ActivationFunctionType

    nc = bacc.Bacc("TRN2", target_bir_lowering=False, debug=False)

    xq = nc.dram_tensor("xq", [C, HW], bf16, kind="ExternalInput")
    xk = nc.dram_tensor("xk", [C, HW], bf16, kind="ExternalInput")
    xv = nc.dram_tensor("xv", [C, HW], bf16, kind="ExternalInput")
    # packed weights: cols 0:128 = wqT, 128:256 = wkT, 256 = bq, 257 = bk
    wpk = nc.dram_tensor("wpk", [C, 2 * C + 2], f32, kind="ExternalInput")
    out = nc.dram_tensor("out", [C, HW], f32, kind="ExternalOutput")

    inv_sqrt_hw = 1.0 / float(np.sqrt(HW))

    with tile.TileContext(nc) as tc:
        with (
            tc.tile_pool(name="const", bufs=1) as cpool,
            tc.tile_pool(name="stage", bufs=1) as spool,
            tc.tile_pool(name="expp", bufs=12) as epool,
            tc.tile_pool(name="fin", bufs=1) as fpool,
            tc.tile_pool(name="ps_s", bufs=2, space="PSUM") as pss,
            tc.tile_pool(name="ps_o", bufs=3, space="PSUM") as pso,
            tc.tile_pool(name="ps_t", bufs=1, space="PSUM") as pst,
        ):
            # ---------- constants / weights ----------
            wpk_sb = cpool.tile([C, 2 * C + 2], f32, name="wpk_sb")
            wq_b = cpool.tile([C, C], bf16, name="wq_b")
            wk_b = cpool.tile([C, C], bf16, name="wk_b")
            ident_b = cpool.tile([P, P], bf16, name="ident_b")
            zeros_b = cpool.tile([P, 3 * 129], bf16, name="zeros_b")
            scr_sb = cpool.tile([P, 12], f32, name="scr_sb")
            wq_f = wpk_sb[:, 0:C]
            wk_f = wpk_sb[:, C:2 * C]
            bq_v = wpk_sb[:, 2 * C:2 * C + 1]
            bk_v = wpk_sb[:, 2 * C + 1:2 * C + 2]

            # ---------- staging (all bf16) ----------
            xq_sb = spool.tile([C, HW], bf16, name="xq_sb")
            xk_sb = spool.tile([C, HW], bf16, name="xk_sb")
            xv_sb = spool.tile([C, HW], bf16, name="xv_sb")
            q_bf = spool.tile([C, HW], bf16, name="q_bf")
            k_bf = spool.tile([C, HW], bf16, name="k_bf")
            vt = spool.tile([P, NTK, VT_STRIDE], bf16, name="vt")
            vtd = spool.tile([P, NTK, P], bf16, name="vtd")

            # critical head DMAs: weights + q head on sync, k head on the
            # scalar HWDGE ring (parallel issue)
            nc.sync.dma_start(wpk_sb[:], wpk[:])
            nc.sync.dma_start(xq_sb[:, 0:TQ], xq[:, 0:TQ])
            nc.sync.dma_start(xq_sb[:, TQ:1024], xq[:, TQ:1024])
            nc.scalar.dma_start(xk_sb[:, 0:1024], xk[:, 0:1024])
            nc.scalar.dma_start(xk_sb[:, 1024:2048], xk[:, 1024:2048])

            make_identity(nc, ident_b)
            nc.gpsimd.memset(zeros_b[:], 0.0)
            nc.gpsimd.memset(vt[:, :, 128:129], 1.0)


            # gates: gpsimd reads of the bulk destination regions, each with
            # a REAL RAW dependency on a head-critical column (the scalar
            # operand). Bulk DMA writes must wait for these reads (WAR) -
            # the tile scheduler cannot reorder them earlier.
            nc.gpsimd.tensor_tensor(scr_sb[:, 0:4],
                                    xq_sb[:, 1023:4096:1024],
                                    xq_sb[:, 1023:1024].broadcast_to((P, 4)),
                                    mybir.AluOpType.add)
            nc.gpsimd.tensor_tensor(scr_sb[:, 4:6],
                                    xk_sb[:, 3071:4096:1024],
                                    xk_sb[:, 2047:2048].broadcast_to((P, 2)),
                                    mybir.AluOpType.add)
            nc.gpsimd.tensor_tensor(scr_sb[:, 8:12],
                                    xv_sb[:, 0:4096:1024],
                                    xq_sb[:, 1023:1024].broadcast_to((P, 4)),
                                    mybir.AluOpType.add)

            # bulk DMAs (sync ring), deadline-ordered (k slices feed the
            # iter-2/4 projection trickles, so they go first)
            def vt_slab(s):
                # 16-bit DMA transpose; XBAR path needs a dense dst - DVE
                # copies bridge to the ones-column layout below. On the
                # sync ring (on the scalar ring they'd block the exp stream)
                nc.sync.dma_start(vtd[:, 8 * s:8 * (s + 1), :],
                                  xv_sb[:, 1024 * s:1024 * (s + 1)],
                                  transpose=True)

            nc.sync.dma_start(xv_sb[:, 0:1024], xv[:, 0:1024])
            nc.sync.dma_start(xv_sb[:, 1024:2048], xv[:, 1024:2048])
            nc.sync.dma_start(xk_sb[:, 2048:HW], xk[:, 2048:HW])
            nc.sync.dma_start(xv_sb[:, 2048:HW], xv[:, 2048:HW])
            nc.sync.dma_start(xq_sb[:, 1024:2560], xq[:, 1024:2560])
            nc.sync.dma_start(xq_sb[:, 2560:HW], xq[:, 2560:HW])
            # transpose issues LAST: each occupies the ring ~1.3us and waits
            # inline on its source slab - they must not delay bulk issues.
            # Slab 0 is built by PE transposes instead (trickled over iters
            # 0-7, where no PV batches compete for the PE yet) because the
            # DMA-transposed slabs only land ~10us into the stream.
            for s in range(1, 4):
                vt_slab(s)

            # ---------- head projections (q chunk 0, k blocks 0-4) ----------
            nc.vector.tensor_copy(wq_b[:], wq_f)
            nc.vector.tensor_copy(wk_b[:], wk_f)
            qps = []
            for j in range(2):
                qp = pss.tile([P, TQ], f32, name="qp", tag="ps")
                nc.tensor.matmul(qp[:], wq_b[:],
                                 xq_sb[:, j * TQ:(j + 1) * TQ],
                                 start=True, stop=True)
                qps.append(qp)
            # one k matmul for blocks 0-4; q bias 1 on the (idle) ScalarE
            # so the serial DVE bias chain doesn't gate the first scores
            kp_a = pss.tile([P, TQC], f32, name="s_ps", tag="ps")
            nc.tensor.matmul(kp_a[:, 0:TQ], wk_b[:], xk_sb[:, 0:TQ],
                             start=True, stop=True)
            nc.tensor.matmul(kp_a[:, TQ:640], wk_b[:], xk_sb[:, TQ:640],
                             start=True, stop=True)
            nc.vector.tensor_scalar_add(q_bf[:, 0:TQ], qps[0][:], bq_v)
            nc.scalar.add(q_bf[:, TQ:2 * TQ], qps[1][:], bq_v)
            nc.vector.tensor_scalar_add(k_bf[:, 0:640], kp_a[:, 0:640],
                                        bk_v)

            # ---------- steady-state emission helpers ----------
            # For chunks 1-3 the last SCH columns of each exp tile are
            # computed on the DVE via the Schraudolph bit-trick: the bf16
            # pattern of exp(s/64) ~= high 16 bits of int32(s*a + b), and
            # int16(s*a/2^16 + b/2^16) equals exactly that in ONE fused
            # multiply-add-convert (softmax cancels the approximation's
            # multiplicative bias; verified end-to-end in numpy).
            SCH = 128
            sch_a = (2.0 ** 23 / np.log(2.0)) / 64.0 / 65536.0
            sch_b = (127 * 2 ** 23 - 486411 + 32768) / 65536.0
            i16 = mybir.dt.int16

            def emit_s_exp(g):
                chunk, blk = divmod(g, NTK)
                s_ps = pss.tile([P, TQC], f32, name="s_ps", tag="ps")
                for h in range(2):
                    nc.tensor.matmul(
                        s_ps[:, h * TQ:(h + 1) * TQ],
                        k_bf[:, blk * P:(blk + 1) * P],
                        q_bf[:, chunk * TQC + h * TQ:
                             chunk * TQC + (h + 1) * TQ],
                        start=True, stop=True)
                e_sb = epool.tile([P, TQC], bf16, name="e_sb", tag="exp")
                fd = TQC if g < 8 else TQC - SCH
                nc.scalar.activation(e_sb[:, 0:fd], s_ps[:, 0:fd], AF.Exp,
                                     scale=inv_sqrt_hw)
                if fd < TQC:
                    nc.vector.tensor_scalar(
                        e_sb[:, fd:TQC].bitcast(i16), s_ps[:, fd:TQC],
                        sch_a, sch_b,
                        mybir.AluOpType.mult, mybir.AluOpType.add)
                return e_sb

            o_tiles = [None]
            e_ring: dict = {}

            def emit_zero_init():
                o_tiles[0] = [pso.tile([P, 3, 129], f32, name="o_ps", tag="o")
                              for _ in range(3)]

            def emit_pv(j):
                blk = j % NTK
                e_sb = e_ring.pop(j)
                for u in range(8):
                    # first write to each bank carries start=True: it clears
                    # the whole bank's has_written bits, so the other packed
                    # accumulators' first start=False writes store (not add)
                    nc.tensor.matmul(
                        o_tiles[0][u // 3][:, u % 3, 0:129],
                        e_sb[:, u * P:(u + 1) * P],
                        vt[:, blk, 0:129],
                        start=(blk == 0 and u % 3 == 0),
                        stop=(blk == NTK - 1),
                        skip_group_check=True)

            # finalize state
            recs = [None, None, None]
            anbs = [None]
            obs = [None]

            def emit_pass1(chunk):
                ot = o_tiles[0]
                last = chunk == NCHUNK - 1
                for t in range(3):
                    rec = fpool.tile([P, 3], f32, name="rec", tag="rec",
                                     bufs=6)
                    nc.vector.reciprocal(rec[:], ot[t][:, :, 128])
                    recs[t] = rec
                anbs[0] = fpool.tile([P, 8, P], bf16, name="anb", tag="an",
                                     bufs=2)
                for u in range(8):
                    if last and u % 2 == 0:
                        # ScalarE is idle after the exp stream: normalize
                        # half the units there so the tail isn't DVE-serial
                        nc.scalar.activation(
                            anbs[0][:, u, :], ot[u // 3][:, u % 3, 0:128],
                            AF.Copy, scale=recs[u // 3][:, u % 3:u % 3 + 1])
                    else:
                        nc.vector.tensor_scalar_mul(
                            anbs[0][:, u, :], ot[u // 3][:, u % 3, 0:128],
                            recs[u // 3][:, u % 3:u % 3 + 1])

            # chunks 0-2: one DMA-transpose + one wide DVE add + one store
            # (zero PE / minimal DVE mid-stream); last chunk: per-unit PE
            # pipeline for low tail latency
            def emit_fin_dmat(chunk):
                obs[0] = (fpool.tile([P, 8, P], bf16, name="anT", tag="anT",
                                     bufs=2),
                          fpool.tile([P, TQC], f32, name="ob", tag="ob",
                                     bufs=2), anbs[0])
                nc.sync.dma_start(obs[0][0][:, :, :],
                                  obs[0][2][:].bitcast(bf16),
                                  transpose=True)

            def emit_fin_add(chunk):
                anT, ob, _ = obs[0]
                nc.vector.tensor_add(ob[:], anT[:, :, :],
                                     q_bf[:, chunk * TQC:(chunk + 1) * TQC])
                nc.sync.dma_start(out[:, chunk * TQC:(chunk + 1) * TQC],
                                  ob[:])

            def emit_pass2(chunk, u):
                tq0 = chunk * TQC + u * P
                tp = pst.tile([P, P], bf16, name="tpb", tag="t")
                nc.tensor.transpose(tp[:], anbs[0][:, u, :], ident_b[:])
                if u == 0:
                    obs[0] = fpool.tile([P, TQC], f32, name="ob", tag="ob",
                                        bufs=2)
                nc.vector.tensor_add(obs[0][:, u * P:(u + 1) * P], tp[:],
                                     q_bf[:, tq0:tq0 + P])
                if u == 3:
                    nc.sync.dma_start(out[:, chunk * TQC:chunk * TQC + 4 * P],
                                      obs[0][:, 0:4 * P])
                if u == 7:
                    nc.sync.dma_start(
                        out[:, chunk * TQC + 4 * P:(chunk + 1) * TQC],
                        obs[0][:, 4 * P:])

            def emit_kproj(c0, c1):
                kp = pst.tile([P, TQ], f32, name="tp", tag="t")
                nc.tensor.matmul(kp[:, 0:c1 - c0], wk_b[:], xk_sb[:, c0:c1],
                                 start=True, stop=True)
                nc.vector.tensor_scalar_add(k_bf[:, c0:c1],
                                            kp[:, 0:c1 - c0], bk_v)

            def emit_qproj(c0, c1):
                qp = pst.tile([P, TQ], f32, name="tp", tag="t")
                nc.tensor.matmul(qp[:, 0:c1 - c0], wq_b[:], xq_sb[:, c0:c1],
                                 start=True, stop=True)
                nc.vector.tensor_scalar_add(q_bf[:, c0:c1],
                                            qp[:, 0:c1 - c0], bq_v)

            # ---------- schedule tables ----------
            # V emission iteration for block j
            v_iter: dict = {}
            for j in range(NBLK):
                c, b = divmod(j, NTK)
                if c == 0:
                    # vT slabs land late in the head: start chunk 0's PV
                    # 9 blocks behind and taper back to the steady lag
                    it = j + (9 if b < 8 else 8 if b < 16 else
                              7 if b < 20 else 6 if b < 24 else LAG)
                elif b < 6:
                    it = 32 * c + 8 + b // 2   # boundary pause + catch-up
                else:
                    it = j + LAG          # steady lag
                v_iter.setdefault(it, []).append(j)

            trickle: dict = {}

            def add_trickle(it, fn):
                trickle.setdefault(it, []).append(fn)

            def emit_vt_pe(b):
                tp = pst.tile([P, P], bf16, name="tpb", tag="t")
                nc.tensor.transpose(tp[:], xv_sb[:, b * P:(b + 1) * P],
                                    ident_b[:])
                nc.vector.tensor_copy(vt[:, b, 0:128], tp[:])

            # k projections FIRST within an iteration (scores have no slack;
            # vT consumers lag far behind)
            add_trickle(0, (lambda: emit_kproj(640, 1024)))
            for s in range(6):
                c0 = 1024 + s * TQ
                add_trickle(3 + 2 * s,
                            (lambda c0=c0: emit_kproj(c0, c0 + TQ)))
            # vT blocks 0-7: PE transpose path; 8-31: copies from the
            # DMA-transposed dense slabs
            for b in range(8):
                add_trickle(b + 6, (lambda b=b: emit_vt_pe(b)))
            for b in range(8, NTK):
                add_trickle(b + 1, (lambda b=b: nc.vector.tensor_copy(
                    vt[:, b, 0:128], vtd[:, b, :])))
            # q projections for chunks 1..3
            for s in range(2, 8):
                c0 = s * TQ
                k = s // 2  # chunk index
                add_trickle(32 * k - 6 + (s % 2) * 2,
                            (lambda c0=c0: emit_qproj(c0, c0 + TQ)))
            # pass2 units for chunks 0..2 trickled into the next chunk
            for c in range(NCHUNK - 1):
                for u in range(8):
                    add_trickle(32 * (c + 1) + 6 + 2 * u,
                                (lambda c=c, u=u: emit_pass2(c, u)))

            # ---------- main loop ----------
            emit_zero_init()
            for g in range(max(v_iter) + 1):
                if g < NBLK:
                    e_ring[g] = emit_s_exp(g)
                for fn in trickle.get(g, ()):
                    fn()
                for j in v_iter.get(g, ()):
                    if j % NTK == 0 and j > 0:
                        emit_zero_init()
                    emit_pv(j)
                    if j % NTK == NTK - 1:
                        emit_pass1(j // NTK)
            # tail: finalize last chunk
            for u in range(8):
                emit_pass2(NCHUNK - 1, u)

    nc.finalize()
    return nc


def kernel(query_img, key_img, value_img, Wq, bq, Wk, bk):
    import ml_dtypes
    from concourse.bass_utils import run_bass_kernel_spmd

    global LAST_RESULTS

    bft = ml_dtypes.bfloat16
    query_img = np.asarray(query_img, dtype=np.float32)
    key_img = np.asarray(key_img, dtype=np.float32)
    value_img = np.asarray(value_img, dtype=np.float32)
    wqT = np.asarray(Wq, dtype=np.float32).T
    wkT = np.asarray(Wk, dtype=np.float32).T
    wpk = np.ascontiguousarray(np.concatenate(
        [wqT, wkT,
         np.asarray(bq, dtype=np.float32).reshape(C, 1),
         np.asarray(bk, dtype=np.float32).reshape(C, 1)], axis=1))

    if "nc" not in _CACHE:
        _CACHE["nc"] = _build_kernel()
    nc = _CACHE["nc"]

    in_maps = []
    for b in range(B):
        in_maps.append({
            "xq": np.ascontiguousarray(
                query_img[b].reshape(C, HW).astype(bft)),
            "xk": np.ascontiguousarray(
                key_img[b].reshape(C, HW).astype(bft)),
            "xv": np.ascontiguousarray(
                value_img[b].reshape(C, HW).astype(bft)),
            "wpk": wpk,
        })

    trace = os.environ.get("KERNEL_TRACE", "0") == "1"
    res = run_bass_kernel_spmd(nc, in_maps, core_ids=list(range(B)),
                               trace=trace)
    LAST_RESULTS = res
    out = np.stack([res.results[b]["out"].reshape(C, H, W) for b in range(B)])
    return out.astype(np.float32)


# revision 38
# speedup vs baseline: 1.0097x; 1.0097x over previous
"""CrossAttention2D Trainium2 Bass kernel.

Problem (per batch item b, C=128, HW=64*64=4096):
    q = Wq @ xq + bq            # [C, HW]   (1x1 conv == GEMM)
    k = Wk @ xk + bk            # [C, HW]
    S = (q^T k) / sqrt(HW)      # [HW, HW]
    A = softmax(S, axis=-1)
    out = (A @ v^T)^T + q       # [C, HW],  v = xv
Sharding: data-parallel over batch B=8 -> one batch item per NeuronCore.

Per-core schedule (ACT exp stream is the bottleneck: 128 ops x ~1.06us):
  - Inputs are pre-cast to bf16 on the HOST (halves DMA bytes, kills all
    on-device input casts; projections were bf16 anyway).
  - Scores computed TRANSPOSED: S^T tiles [tk=128, tq=1024], exp on
    ScalarE with FD=1024 (2 PSUM banks, ring of 2).
  - Software pipeline per iteration g: emit scores/exp for block g FIRST,
    then trickled work (projections, finalize), then PV batches that lag
    LAG blocks behind - so ScalarE never waits.
  - PV: out_ext[tq,129] += expS^T_slice^T @ vT_ext (ones column gives the
    softmax denominator for free), accumulated over 32 key blocks in 3
    PSUM banks (3 accumulators packed per bank).
  - Last SCH columns of each exp tile for chunks 1-3 computed on the DVE
    via the Schraudolph bit-trick (one fused mul-add-convert to int16 ==
    the bf16 pattern of exp) - offloads ~12% of the exp stream from the
    bottleneck ScalarE; softmax cancels the approximation bias.
  - vT: slab 0 by PE transposes (PE is idle pre-PV), slabs 1-3 by 16-bit
    DMA-transpose into a dense buffer, DVE copies bridge to the
    ones-column layout. Chunk 0's PV lag starts at 9 (slabs land late).
  - Chunk boundaries: V emission pauses 3 iterations so pass1 (recip +
    normalize on DVE) can drain the o-banks before the next chunk's
    first PV (start=True on the first write per bank replaces zero-init
    matmuls); V catches back up with double-emission iterations.
  - Bulk input DMAs are held back behind the head-critical transfers via
    gpsimd WAR-reads of their destination regions (a plain emission-order
    gate gets reordered by the tile scheduler).
  - Tail: finalize of the last chunk splits normalize between the (idle)
    ScalarE and DVE; chunked output stores.
"""

import os
import numpy as np

B, C, H, W = 8, 128, 64, 64
HW = H * W            # 4096
P = 128
TQ = 512              # one score matmul / one PSUM bank
TQC = 1024            # query-token chunk (2 banks -> one FD=1024 exp)
NCHUNK = HW // TQC    # 4
NTK = HW // P         # 32 key blocks
NBLK = NCHUNK * NTK   # 128 score/exp groups
VT_STRIDE = 130       # 129 used + 1 pad (4B alignment per block)
LAG = 5               # PV lags scores/exp by this many blocks

_CACHE: dict = {}
LAST_RESULTS = None   # BassKernelResults of the most recent run (for test.py)


def _build_kernel():
    import concourse.tile as tile
    from concourse import bacc, mybir
    from concourse.masks import make_identity

    f32 = mybir.dt.float32
    bf16 = mybir.dt.bfloat16
    AF = mybir.ActivationFunctionType

    nc = bacc.Bacc("TRN2", target_bir_lowering=False, debug=False)

    xq = nc.dram_tensor("xq", [C, HW], bf16, kind="ExternalInput")
    xk = nc.dram_tensor("xk", [C, HW], bf16, kind="ExternalInput")
    xv = nc.dram_tensor("xv", [C, HW], bf16, kind="ExternalInput")
    # packed weights: cols 0:128 = wqT, 128:256 = wkT, 256 = bq, 257 = bk
    wpk = nc.dram_tensor("wpk", [C, 2 * C + 2], f32, kind="ExternalInput")
    out = nc.dram_tensor("out", [C, HW], f32, kind="ExternalOutput")

    inv_sqrt_hw = 1.0 / float(np.sqrt(HW))

    with tile.TileContext(nc) as tc:
        with (
            tc.tile_pool(name="const", bufs=1) as cpool,
            tc.tile_pool(name="stage", bufs=1) as spool,
            tc.tile_pool(name="expp", bufs=12) as epool,
            tc.tile_pool(name="fin", bufs=1) as fpool,
            tc.tile_pool(name="ps_s", bufs=2, space="PSUM") as pss,
            tc.tile_pool(name="ps_o", bufs=3, space="PSUM") as pso,
            tc.tile_pool(name="ps_t", bufs=1, space="PSUM") as pst,
        ):
            # ---------- constants / weights ----------
            wpk_sb = cpool.tile([C, 2 * C + 2], f32, name="wpk_sb")
            wq_b = cpool.tile([C, C], bf16, name="wq_b")
            wk_b = cpool.tile([C, C], bf16, name="wk_b")
            ident_b = cpool.tile([P, P], bf16, name="ident_b")
            zeros_b = cpool.tile([P, 3 * 129], bf16, name="zeros_b")
            scr_sb = cpool.tile([P, 12], f32, name="scr_sb")
            wq_f = wpk_sb[:, 0:C]
            wk_f = wpk_sb[:, C:2 * C]
            bq_v = wpk_sb[:, 2 * C:2 * C + 1]
            bk_v = wpk_sb[:, 2 * C + 1:2 * C + 2]

            # ---------- staging (all bf16) ----------
            xq_sb = spool.tile([C, HW], bf16, name="xq_sb")
            xk_sb = spool.tile([C, HW], bf16, name="xk_sb")
            xv_sb = spool.tile([C, HW], bf16, name="xv_sb")
            q_bf = spool.tile([C, HW], bf16, name="q_bf")
            k_bf = spool.tile([C, HW], bf16, name="k_bf")
            vt = spool.tile([P, NTK, VT_STRIDE], bf16, name="vt")
            vtd = spool.tile([P, NTK, P], bf16, name="vtd")

            # critical head DMAs: weights + q head on sync, k head on the
            # scalar HWDGE ring (parallel issue)
            nc.sync.dma_start(wpk_sb[:], wpk[:])
            nc.sync.dma_start(xq_sb[:, 0:TQ], xq[:, 0:TQ])
            nc.sync.dma_start(xq_sb[:, TQ:1024], xq[:, TQ:1024])
            nc.scalar.dma_start(xk_sb[:, 0:1024], xk[:, 0:1024])
            nc.scalar.dma_start(xk_sb[:, 1024:2048], xk[:, 1024:2048])

            make_identity(nc, ident_b)
            nc.gpsimd.memset(zeros_b[:], 0.0)
            nc.gpsimd.memset(vt[:, :, 128:129], 1.0)


            # gates: gpsimd reads of the bulk destination regions, each with
            # a REAL RAW dependency on a head-critical column (the scalar
            # operand). Bulk DMA writes must wait for these reads (WAR) -
            # the tile scheduler cannot reorder them earlier.
            nc.gpsimd.tensor_tensor(scr_sb[:, 0:4],
                                    xq_sb[:, 1023:4096:1024],
                                    xq_sb[:, 1023:1024].broadcast_to((P, 4)),
                                    mybir.AluOpType.add)
            nc.gpsimd.tensor_tensor(scr_sb[:, 4:6],
                                    xk_sb[:, 3071:4096:1024],
                                    xk_sb[:, 2047:2048].broadcast_to((P, 2)),
                                    mybir.AluOpType.add)
            nc.gpsimd.tensor_tensor(scr_sb[:, 8:12],
                                    xv_sb[:, 0:4096:1024],
                                    xq_sb[:, 1023:1024].broadcast_to((P, 4)),
                                    mybir.AluOpType.add)

            # bulk DMAs (sync ring), deadline-ordered (k slices feed the
            # iter-2/4 projection trickles, so they go first)
            def vt_slab(s):
                # 16-bit DMA transpose; XBAR path needs a dense dst - DVE
                # copies bridge to the ones-column layout below. On the
                # sync ring (on the scalar ring they'd block the exp stream)
                nc.sync.dma_start(vtd[:, 8 * s:8 * (s + 1), :],
                                  xv_sb[:, 1024 * s:1024 * (s + 1)],
                                  transpose=True)

            nc.sync.dma_start(xv_sb[:, 0:1024], xv[:, 0:1024])
            nc.sync.dma_start(xv_sb[:, 1024:2048], xv[:, 1024:2048])
            nc.sync.dma_start(xk_sb[:, 2048:HW], xk[:, 2048:HW])
            nc.sync.dma_start(xv_sb[:, 2048:HW], xv[:, 2048:HW])
            nc.sync.dma_start(xq_sb[:, 1024:2560], xq[:, 1024:2560])
            nc.sync.dma_start(xq_sb[:, 2560:HW], xq[:, 2560:HW])
            # transpose issues LAST: each occupies the ring ~1.3us and waits
            # inline on its source slab - they must not delay bulk issues.
            # Slab 0 is built by PE transposes instead (trickled over iters
            # 0-7, where no PV batches compete for the PE yet) because the
            # DMA-transposed slabs only land ~10us into the stream.
            for s in range(1, 4):
                vt_slab(s)

            # ---------- head projections (q chunk 0, k blocks 0-4) ----------
            nc.vector.tensor_copy(wq_b[:], wq_f)
            nc.vector.tensor_copy(wk_b[:], wk_f)
            qps = []
            for j in range(2):
                qp = pss.tile([P, TQ], f32, name="qp", tag="ps")
                nc.tensor.matmul(qp[:], wq_b[:],
                                 xq_sb[:, j * TQ:(j + 1) * TQ],
                                 start=True, stop=True)
                qps.append(qp)
            # one k matmul for blocks 0-4; q bias 1 on the (idle) ScalarE
            # so the serial DVE bias chain doesn't gate the first scores
            kp_a = pss.tile([P, TQC], f32, name="s_ps", tag="ps")
            nc.tensor.matmul(kp_a[:, 0:TQ], wk_b[:], xk_sb[:, 0:TQ],
                             start=True, stop=True)
            nc.tensor.matmul(kp_a[:, TQ:640], wk_b[:], xk_sb[:, TQ:640],
                             start=True, stop=True)
            nc.vector.tensor_scalar_add(q_bf[:, 0:TQ], qps[0][:], bq_v)
            nc.scalar.add(q_bf[:, TQ:2 * TQ], qps[1][:], bq_v)
            nc.vector.tensor_scalar_add(k_bf[:, 0:640], kp_a[:, 0:640],
                                        bk_v)

            # ---------- steady-state emission helpers ----------
            # For chunks 1-3 the last SCH columns of each exp tile are
            # computed on the DVE via the Schraudolph bit-trick: the bf16
            # pattern of exp(s/64) ~= high 16 bits of int32(s*a + b), and
            # int16(s*a/2^16 + b/2^16) equals exactly that in ONE fused
            # multiply-add-convert (softmax cancels the approximation's
            # multiplicative bias; verified end-to-end in numpy).
            SCH = 128
            sch_a = (2.0 ** 23 / np.log(2.0)) / 64.0 / 65536.0
            sch_b = (127 * 2 ** 23 - 486411 + 32768) / 65536.0
            i16 = mybir.dt.int16

            def emit_s_exp(g):
                chunk, blk = divmod(g, NTK)
                s_ps = pss.tile([P, TQC], f32, name="s_ps", tag="ps")
                for h in range(2):
                    nc.tensor.matmul(
                        s_ps[:, h * TQ:(h + 1) * TQ],
                        k_bf[:, blk * P:(blk + 1) * P],
                        q_bf[:, chunk * TQC + h * TQ:
                             chunk * TQC + (h + 1) * TQ],
                        start=True, stop=True)
                e_sb = epool.tile([P, TQC], bf16, name="e_sb", tag="exp")
                fd = TQC if g < 8 else TQC - SCH
                nc.scalar.activation(e_sb[:, 0:fd], s_ps[:, 0:fd], AF.Exp,
                                     scale=inv_sqrt_hw)
                if fd < TQC:
                    nc.vector.tensor_scalar(
                        e_sb[:, fd:TQC].bitcast(i16), s_ps[:, fd:TQC],
                        sch_a, sch_b,
                        mybir.AluOpType.mult, mybir.AluOpType.add)
                return e_sb

            o_tiles = [None]
            e_ring: dict = {}

            def emit_zero_init():
                o_tiles[0] = [pso.tile([P, 3, 129], f32, name="o_ps", tag="o")
                              for _ in range(3)]

            def emit_pv(j):
                blk = j % NTK
                e_sb = e_ring.pop(j)
                for u in range(8):
                    # first write to each bank carries start=True: it clears
                    # the whole bank's has_written bits, so the other packed
                    # accumulators' first start=False writes store (not add)
                    nc.tensor.matmul(
                        o_tiles[0][u // 3][:, u % 3, 0:129],
                        e_sb[:, u * P:(u + 1) * P],
                        vt[:, blk, 0:129],
                        start=(blk == 0 and u % 3 == 0),
                        stop=(blk == NTK - 1),
                        skip_group_check=True)

            # finalize state
            recs = [None, None, None]
            anbs = [None]
            obs = [None]

            def emit_pass1(chunk):
                ot = o_tiles[0]
                last = chunk == NCHUNK - 1
                for t in range(3):
                    rec = fpool.tile([P, 3], f32, name="rec", tag="rec",
                                     bufs=6)
                    nc.vector.reciprocal(rec[:], ot[t][:, :, 128])
                    recs[t] = rec
                anbs[0] = fpool.tile([P, 8, P], bf16, name="anb", tag="an",
                                     bufs=2)
                for u in range(8):
                    if last and u % 2 == 0:
                        # ScalarE is idle after the exp stream: normalize
                        # half the units there so the tail isn't DVE-serial
                        nc.scalar.activation(
                            anbs[0][:, u, :], ot[u // 3][:, u % 3, 0:128],
                            AF.Copy, scale=recs[u // 3][:, u % 3:u % 3 + 1])
                    else:
                        nc.vector.tensor_scalar_mul(
                            anbs[0][:, u, :], ot[u // 3][:, u % 3, 0:128],
                            recs[u // 3][:, u % 3:u % 3 + 1])

            # chunks 0-2: one DMA-transpose + one wide DVE add + one store
            # (zero PE / minimal DVE mid-stream); last chunk: per-unit PE
            # pipeline for low tail latency
            def emit_fin_dmat(chunk):
                obs[0] = (fpool.tile([P, 8, P], bf16, name="anT", tag="anT",
                                     bufs=2),
                          fpool.tile([P, TQC], f32, name="ob", tag="ob",
                                     bufs=2), anbs[0])
                nc.sync.dma_start(obs[0][0][:, :, :],
                                  obs[0][2][:].bitcast(bf16),
                                  transpose=True)

            def emit_fin_add(chunk):
                anT, ob, _ = obs[0]
                nc.vector.tensor_add(ob[:], anT[:, :, :],
                                     q_bf[:, chunk * TQC:(chunk + 1) * TQC])
                nc.sync.dma_start(out[:, chunk * TQC:(chunk + 1) * TQC],
                                  ob[:])

            tps = [None] * 8

            def emit_pass2_t(chunk, u):
                tp = pst.tile([P, P], bf16, name="tpb", tag="t")
                nc.tensor.transpose(tp[:], anbs[0][:, u, :], ident_b[:])
                tps[u] = tp

            def emit_pass2_a(chunk, u):
                tq0 = chunk * TQC + u * P
                if u == 0:
                    obs[0] = fpool.tile([P, TQC], f32, name="ob", tag="ob",
                                        bufs=2)
                nc.vector.tensor_add(obs[0][:, u * P:(u + 1) * P],
                                     tps[u][:], q_bf[:, tq0:tq0 + P])
                if u == 3:
                    nc.sync.dma_start(out[:, chunk * TQC:chunk * TQC + 4 * P],
                                      obs[0][:, 0:4 * P])
                if u == 7:
                    nc.sync.dma_start(
                        out[:, chunk * TQC + 4 * P:(chunk + 1) * TQC],
                        obs[0][:, 4 * P:])

            def emit_pass2(chunk, u):
                tq0 = chunk * TQC + u * P
                tp = pst.tile([P, P], bf16, name="tpb", tag="t")
                nc.tensor.transpose(tp[:], anbs[0][:, u, :], ident_b[:])
                if u == 0:
                    obs[0] = fpool.tile([P, TQC], f32, name="ob", tag="ob",
                                        bufs=2)
                nc.vector.tensor_add(obs[0][:, u * P:(u + 1) * P], tp[:],
                                     q_bf[:, tq0:tq0 + P])
                if u == 3:
                    nc.sync.dma_start(out[:, chunk * TQC:chunk * TQC + 4 * P],
                                      obs[0][:, 0:4 * P])
                if u == 7:
                    nc.sync.dma_start(
                        out[:, chunk * TQC + 4 * P:(chunk + 1) * TQC],
                        obs[0][:, 4 * P:])

            def emit_kproj(c0, c1):
                kp = pst.tile([P, TQ], f32, name="tp", tag="t")
                nc.tensor.matmul(kp[:, 0:c1 - c0], wk_b[:], xk_sb[:, c0:c1],
                                 start=True, stop=True)
                nc.vector.tensor_scalar_add(k_bf[:, c0:c1],
                                            kp[:, 0:c1 - c0], bk_v)

            def emit_qproj(c0, c1):
                qp = pst.tile([P, TQ], f32, name="tp", tag="t")
                nc.tensor.matmul(qp[:, 0:c1 - c0], wq_b[:], xq_sb[:, c0:c1],
                                 start=True, stop=True)
                nc.vector.tensor_scalar_add(q_bf[:, c0:c1],
                                            qp[:, 0:c1 - c0], bq_v)

            # ---------- schedule tables ----------
            # V emission iteration for block j
            v_iter: dict = {}
            for j in range(NBLK):
                c, b = divmod(j, NTK)
                if c == 0:
                    # vT slabs land late in the head: start chunk 0's PV
                    # 9 blocks behind and taper back to the steady lag
                    it = j + (9 if b < 8 else 8 if b < 16 else
                              7 if b < 20 else 6 if b < 24 else LAG)
                elif b < 6:
                    it = 32 * c + 8 + b // 2   # boundary pause + catch-up
                else:
                    it = j + LAG          # steady lag
                v_iter.setdefault(it, []).append(j)

            trickle: dict = {}

            def add_trickle(it, fn):
                trickle.setdefault(it, []).append(fn)

            def emit_vt_pe(b):
                tp = pst.tile([P, P], bf16, name="tpb", tag="t")
                nc.tensor.transpose(tp[:], xv_sb[:, b * P:(b + 1) * P],
                                    ident_b[:])
                nc.vector.tensor_copy(vt[:, b, 0:128], tp[:])

            # k projections FIRST within an iteration (scores have no slack;
            # vT consumers lag far behind)
            add_trickle(0, (lambda: emit_kproj(640, 1024)))
            for s in range(6):
                c0 = 1024 + s * TQ
                add_trickle(3 + 2 * s,
                            (lambda c0=c0: emit_kproj(c0, c0 + TQ)))
            # vT blocks 0-7: PE transpose path; 8-31: copies from the
            # DMA-transposed dense slabs
            for b in range(8):
                add_trickle(b + 6, (lambda b=b: emit_vt_pe(b)))
            for b in range(8, NTK):
                add_trickle(b + 1, (lambda b=b: nc.vector.tensor_copy(
                    vt[:, b, 0:128], vtd[:, b, :])))
            # q projections for chunks 1..3
            for s in range(2, 8):
                c0 = s * TQ
                k = s // 2  # chunk index
                add_trickle(32 * k - 6 + (s % 2) * 2,
                            (lambda c0=c0: emit_qproj(c0, c0 + TQ)))
            # pass2 units for chunks 0..2 trickled into the next chunk;
            # the DVE add runs one iteration after its PE transpose so it
            # never head-of-line-blocks the next Schraudolph op
            for c in range(NCHUNK - 1):
                for u in range(8):
                    add_trickle(32 * (c + 1) + 6 + 2 * u,
                                (lambda c=c, u=u: emit_pass2_t(c, u)))
                    add_trickle(32 * (c + 1) + 7 + 2 * u,
                                (lambda c=c, u=u: emit_pass2_a(c, u)))

            # ---------- main loop ----------
            emit_zero_init()
            for g in range(max(v_iter) + 1):
                if g < NBLK:
                    e_ring[g] = emit_s_exp(g)
                for fn in trickle.get(g, ()):
                    fn()
                for j in v_iter.get(g, ()):
                    if j % NTK == 0 and j > 0:
                        emit_zero_init()
                    emit_pv(j)
                    if j % NTK == NTK - 1:
                        emit_pass1(j // NTK)
            # tail: finalize last chunk
            for u in range(8):
                emit_pass2(NCHUNK - 1, u)

    nc.finalize()
    return nc


def kernel(query_img, key_img, value_img, Wq, bq, Wk, bk):
    import ml_dtypes
    from concourse.bass_utils import run_bass_kernel_spmd

    global LAST_RESULTS

    bft = ml_dtypes.bfloat16
    query_img = np.asarray(query_img, dtype=np.float32)
    key_img = np.asarray(key_img, dtype=np.float32)
    value_img = np.asarray(value_img, dtype=np.float32)
    wqT = np.asarray(Wq, dtype=np.float32).T
    wkT = np.asarray(Wk, dtype=np.float32).T
    wpk = np.ascontiguousarray(np.concatenate(
        [wqT, wkT,
         np.asarray(bq, dtype=np.float32).reshape(C, 1),
         np.asarray(bk, dtype=np.float32).reshape(C, 1)], axis=1))

    if "nc" not in _CACHE:
        _CACHE["nc"] = _build_kernel()
    nc = _CACHE["nc"]

    in_maps = []
    for b in range(B):
        in_maps.append({
            "xq": np.ascontiguousarray(
                query_img[b].reshape(C, HW).astype(bft)),
            "xk": np.ascontiguousarray(
                key_img[b].reshape(C, HW).astype(bft)),
            "xv": np.ascontiguousarray(
                value_img[b].reshape(C, HW).astype(bft)),
            "wpk": wpk,
        })

    trace = os.environ.get("KERNEL_TRACE", "0") == "1"
    res = run_bass_kernel_spmd(nc, in_maps, core_ids=list(range(B)),
                               trace=trace)
    LAST_RESULTS = res
    out = np.stack([res.results[b]["out"].reshape(C, H, W) for b in range(B)])
    return out.astype(np.float32)
